# revision 44
# baseline (speedup 1.0000x reference)
"""Trainium2 Bass kernel for nn_MultiHeadedAttention_51737176047655.

Multi-head attention with Music-Transformer relative position bias
(skew trick), B=4, L=1024, D=1024, 16 heads, head_dim=64.

Sharding (8 cores): core = 2*b + hg  -> batch b in [0,4), head-group hg in
[0,2).  Each core computes 8 heads for one batch over the full sequence:
  - Wq/Wk/Wv column-sharded [1024, 512], Wo row-sharded [512, 1024]
  - per-core output is a partial [1024, 1024]; host sums the two
    head-group partials per batch (standard TP unshard) and adds bo.

Device algorithm per core (matmuls bf16 in / f32 PSUM accumulate):
  qT/kT/vT arrive host-transposed [d, l]; projections give qhT/khT
  [d', l] (transposed) and vh [l, d'] (natural, with a ones column per
  head for softmax sums).  QE = qh e^T is computed per head PAIR with
  the two heads' K=64 matmuls adjacent on 64x128 PE row tiles
  (0,0)/(64,0) into different PSUM tiles so they stream concurrently;
  the m-range is clipped to the tri-mask support per l-tile.  Masked QE
  rows (one shifted-tri "slab" multiply per PSUM bank) land in
  persistent stripe buffers whose zero column / zero tails are memset
  once at startup, and a batched DMA writes the padded layout (row
  stride 1025) to a DRAM scratch; reading rows back with stride 1024
  materializes the skewed Srel exactly (the reference's pad+reshape
  trick).  scores are computed TRANSPOSED (scores^T = kh qh^T, head
  pairs packed into PE row groups via tile_position) and Srel^T is
  accumulated into the same PSUM tile by transpose-by-identity matmuls
  (lhsT=srel_chunk, rhs=I), skipping statically-zero 128x128 blocks;
  scores PSUM tiles span 2 banks so one ScalarE exp (scale=1/8) drains
  two j-tiles -> unnormalized attn^T (bf16).  attn@V is ONE K=128
  matmul per (l-half, j-block): the PE drains one output column per
  cycle through its single PSUM write port, so PE time is set by
  output-column count and a K=64 co-streamed split would emit every
  column twice.  The vh ones column lands the softmax denominators Z
  on partition 64 of the 1-bank ctx tile; 1/Z (approx-fast DVE
  reciprocal, Z staged to SBUF first — the custom-DVE op needs a plain
  SBUF operand) is broadcast across 64 partitions by gpsimd
  partition_broadcast and applied by the DVE while packing ctx^T
  head-pairs.
Scheduling: the heads loop is slot-interleaved — each slot emits the
always-ready attn@V matmuls of head h-1 first, then one scores tile of
head h, then one QE l-tile of the pair h+2/h+3 (even h), with skew
reads for head h+1 prefetched mid-head — so a PSUM-rotation stall on
any one stream is absorbed by ready work ahead of it in the in-order
PE queue (keeps the HAM clock gate warm and DMA latency off the
critical path).
DMA queue split: bulk scratch writes ride the gpsimd SWDGE queue,
srel reads + input loads the sync HWDGE queue, and output writes the
scalar/vector HWDGE queues, so no read ever queues behind the 16.8MB
of skew-scratch writes.  The output projection is folded into the
heads loop as per-PAIR bf16 partials (pair p streams through head
2p+3's slots; pair 3 through the tail drain), summed on the host —
there is no serial phase-6 tail.  32 warm-up matmuls at t=0 open the
HAM clock gate while the first input tiles are still in flight.
No max-subtraction in softmax: logits are ~N(0, 1.4^2), far inside
fp32/exp range (validated vs reference at ~1e-6 in fp32 emulation).
"""

import math
import sys

import numpy as np

sys.path.insert(0, "/opt/trn_rl_repo")

import ml_dtypes  # noqa: E402

BF16 = ml_dtypes.bfloat16

# Problem constants (hardcoded per contract)
B = 4
L = 1024
D = 1024
H = 16
HD = 64
H_LOC = 8  # heads per core
DG = 512  # d' columns per core (H_LOC * HD)
NCORES = 8
MAX_SEQ = 2048
PAD = L + 1  # 1025, padded row stride of the skew scratch
FLAT = L * PAD  # 1049600 elements per head scratch

NLT = L // 128  # 8 l-tiles
NDT = D // 128  # 8 contraction tiles
NPAIR = H_LOC // 2  # 4 head pairs


def _build_bass():
    """Build the single-core SPMD Bass program (same program, per-core data)."""
    import concourse.bass as bass
    import concourse.tile as tile
    from concourse import bacc, mybir

    f32 = mybir.dt.float32
    bf16 = mybir.dt.bfloat16
    Exp = mybir.ActivationFunctionType.Exp
    mult = mybir.AluOpType.mult
    addop = mybir.AluOpType.add

    nc = bacc.Bacc(
        "TRN2", target_bir_lowering=False, debug=False, enable_asserts=False
    )

    # ---- kernel I/O (qT/kT/vT are host-transposed [d, l]) ----
    qT_d = nc.declare_dram_parameter("qT", [D, L], bf16, isOutput=False)
    kT_d = nc.declare_dram_parameter("kT", [D, L], bf16, isOutput=False)
    vT_d = nc.declare_dram_parameter("vT", [D, L], bf16, isOutput=False)
    wq_d = nc.declare_dram_parameter("wq", [D, DG], bf16, isOutput=False)
    wk_d = nc.declare_dram_parameter("wk", [D, DG], bf16, isOutput=False)
    wv_d = nc.declare_dram_parameter("wv", [D, DG], bf16, isOutput=False)
    wo_d = nc.declare_dram_parameter("wo", [DG, D], bf16, isOutput=False)
    e2_d = nc.declare_dram_parameter("e2", [128, L], bf16, isOutput=False)
    tri_d = nc.declare_dram_parameter("tri", [128, 128], f32, isOutput=False)
    slab_d = nc.declare_dram_parameter("slab", [128, 640], bf16, isOutput=False)
    # per-PAIR output partials (bf16): host sums the 4 pair partials of each
    # core (and the 2 head-group partials per batch) in f32, so the out
    # projection can stream during the heads loop instead of a serial tail
    out_d = nc.declare_dram_parameter("out", [NPAIR, L, D], bf16, isOutput=True)

    # skew scratch, one padded buffer per local head
    scratch = [nc.dram_tensor(f"skew{h}", [FLAT], bf16) for h in range(H_LOC)]

    # block (lt, jt) of Srel is identically zero unless piece A
    # (j <= 2l-1023) or piece B (l+2 <= j <= 2l+3) intersects it.
    def srel_block_nonzero(lt, jt):
        l1 = 128 * lt + 127
        j0, j1 = 128 * jt, 128 * jt + 127
        a = 2 * l1 - 1023 >= j0
        b = (j1 >= 128 * lt + 2) and (j0 <= 2 * l1 + 3)
        return a or b

    with tile.TileContext(nc) as tc:
        from contextlib import ExitStack

        with ExitStack() as outer:
            # ---------------- persistent pools ----------------
            persist = outer.enter_context(tc.tile_pool(name="persist", bufs=1))
            # projection outputs (live through whole kernel)
            qhT = persist.tile([128, NPAIR, L], bf16)  # [part, pair, l]
            khT = persist.tile([128, NPAIR, L], bf16)
            # vh with ones column per head: [part(j%128), jt, head, 65]
            vh = persist.tile([128, NLT, H_LOC, HD + 1], bf16)
            e2_sb = persist.tile([128, L], bf16)
            slab_sb = persist.tile([128, 640], bf16)
            ctxp = persist.tile([128, NPAIR, L], bf16)  # packed ctx^T per pair
            wo_sb = [
                persist.tile([128, D], bf16, name=f"wo{i}") for i in range(NPAIR)
            ]
            # persistent stripe buffers [hl][lh]: zero col 0 / zero tails are
            # memset once here and never rewritten (QE only touches the data
            # span), so the batched scratch-write DMA always sees zeros there.
            stripes = [
                [
                    persist.tile([128, 4, PAD], bf16, name=f"stripe{hl}{lh}")
                    for lh in range(2)
                ]
                for hl in range(2)
            ]
            # head 0's attn^T halves live in persist so the attT pool can
            # open after the v-input pool closes (LIFO pool order)
            hv0 = [
                persist.tile([128, NLT, 512], bf16, name=f"hv0{i}")
                for i in range(2)
            ]

            nc.vector.memset(vh[:, :, :, HD : HD + 1], 1.0)
            for hl in range(2):
                for lh in range(2):
                    for a in range(4):
                        lt = 4 * lh + a
                        l0 = 128 * lt
                        nc.vector.memset(stripes[hl][lh][:, a, 0:1], 0.0)
                        if l0 + 128 < L:
                            nc.vector.memset(
                                stripes[hl][lh][:, a, 1 + l0 + 128 : PAD], 0.0
                            )

            # ---------------- phase 1+2: loads + projections ----
            with ExitStack() as outer2:
                # scores PSUM tiles are 2 banks wide ([128, 1024] f32): one
                # exp activation then drains two j-tiles, halving ScalarE
                # instruction count (the heads-loop pacing engine)
                sc_ps = outer2.enter_context(
                    tc.tile_pool(name="sc_ps", bufs=3, space="PSUM")
                )
                qe_ps = sc_ps  # QE shares the scores PSUM slots (tag "sc")
                ctx_ps = None  # opened after head 0 (PSUM bank budget)
                attT = None  # opened after tin closes (SBUF budget)
                # srl opens early so head-0 skew reads can issue mid-proj
                srl = outer2.enter_context(tc.tile_pool(name="srl", bufs=4))
                zp = outer2.enter_context(tc.tile_pool(name="zp", bufs=2))

                ident = persist.tile([128, 128], bf16, name="ident")
                from concourse.masks import make_identity

                make_identity(nc, ident)

                # short-lived input pools; q/k inputs close before the heads
                # loop, v inputs + matmul PSUM stay open through head 0 so
                # the vh projection can fill head 0's thin slots
                tinv_blk = ExitStack()
                tinv = tinv_blk.enter_context(tc.tile_pool(name="tinv", bufs=1))
                mm_ps = tinv_blk.enter_context(
                    tc.tile_pool(name="mm_ps", bufs=2, space="PSUM")
                )
                tin_blk = ExitStack()
                tin = tin_blk.enter_context(tc.tile_pool(name="tin", bufs=1))

                # HAM warm-up: the PE clock gate defaults to half rate
                # (K=4/8) and only opens after ~3.4us of sustained activity.
                # The first real matmul can't start until ~3MB of qT/wq lands
                # (~9us), so burn that window with dummy matmuls on garbage
                # SBUF (ctxp is first written at head 1, long after these
                # reads retire) to enter the proj phase at full clock.
                for _ in range(32):
                    wps = mm_ps.tile([128, 512], f32, name="wps", tag="mm")
                    nc.tensor.matmul(
                        wps,
                        ctxp[:, 0, 0:128],
                        ctxp[:, 0, 0:512],
                        start=True,
                        stop=True,
                    )

                qT = [tin.tile([128, L], bf16, name=f"qT{i}") for i in range(NDT)]
                kT = [tin.tile([128, L], bf16, name=f"kT{i}") for i in range(NDT)]
                vT = [tinv.tile([128, L], bf16, name=f"vT{i}") for i in range(NDT)]
                wq_sb = [tin.tile([128, DG], bf16, name=f"wq{i}") for i in range(NDT)]
                wk_sb = [tin.tile([128, DG], bf16, name=f"wk{i}") for i in range(NDT)]
                wv_sb = [
                    tinv.tile([128, DG], bf16, name=f"wv{i}") for i in range(NDT)
                ]

                # q + Wq first so qh projections (and QE) can start early;
                # constants needed later (e2/slab/wo) load after them
                for i in range(NDT):
                    dsl = slice(128 * i, 128 * (i + 1))
                    nc.sync.dma_start(out=qT[i], in_=qT_d[dsl, :])
                    nc.sync.dma_start(out=wq_sb[i], in_=wq_d[dsl, :])
                nc.sync.dma_start(out=e2_sb, in_=e2_d[:, :])
                nc.sync.dma_start(out=slab_sb, in_=slab_d[:, :])
                for i in range(NDT):
                    dsl = slice(128 * i, 128 * (i + 1))
                    nc.sync.dma_start(out=kT[i], in_=kT_d[dsl, :])
                    nc.sync.dma_start(out=wk_sb[i], in_=wk_d[dsl, :])
                # wo is first needed at head 3 (out-proj of pair 0)
                for i in range(NPAIR):
                    nc.sync.dma_start(
                        out=wo_sb[i], in_=wo_d[128 * i : 128 * (i + 1), :]
                    )
                for i in range(NDT):
                    dsl = slice(128 * i, 128 * (i + 1))
                    nc.sync.dma_start(out=vT[i], in_=vT_d[dsl, :])
                    nc.sync.dma_start(out=wv_sb[i], in_=wv_d[dsl, :])

                def proj_pair(w_sb, xT, dst, p):
                    for lh in range(2):
                        ps = mm_ps.tile([128, 512], f32, name="proj_ps", tag="mm")
                        lsl = slice(512 * lh, 512 * (lh + 1))
                        for dt in range(NDT):
                            nc.tensor.matmul(
                                ps,
                                w_sb[dt][:, 128 * p : 128 * (p + 1)],
                                xT[dt][:, lsl],
                                start=(dt == 0),
                                stop=(dt == NDT - 1),
                            )
                        nc.scalar.copy(dst[:, p, lsl], ps)

                def vh_tile(jt):
                    ps = mm_ps.tile([128, 512], f32, name="vh_ps", tag="mm")
                    jsl = slice(128 * jt, 128 * (jt + 1))
                    for dt in range(NDT):
                        nc.tensor.matmul(
                            ps,
                            vT[dt][:, jsl],
                            wv_sb[dt][:, :],
                            start=(dt == 0),
                            stop=(dt == NDT - 1),
                        )
                    # scatter 512 d' columns into per-head [64] slots with one
                    # strided copy (dest stride 65 per head)
                    ps_ap = ps[:, :]
                    ps_view = bass.AP(
                        tensor=ps_ap.tensor,
                        offset=ps_ap.offset,
                        ap=[list(ps_ap.ap)[0], [HD, H_LOC], [1, HD]],
                    )
                    nc.scalar.copy(vh[:, jt, :, 0:HD], ps_view)

                # QE m-range actually needed for l-tile lt (tri mask: m <= l)
                def qe_banks(lt):
                    l0 = 128 * lt
                    need = min(l0 + 128, L)
                    if need <= 512:
                        return [(0, need)]
                    return [(0, 512), (512, need - 512)]

                def qe_part2(hh, s, only_hl=None):
                    """QE + masked stripes for the head pair hh/hh+1, one
                    l-tile (slot s of 8: lh = s//4, a = s%4).  The two heads'
                    K=64 matmuls are adjacent on row tiles (0,0)/(64,0) and
                    write different PSUM tiles, so they stream concurrently.
                    With only_hl set, emits just that head of the pair (one
                    PSUM tile per slot — keeps the shared sc-pool rotation
                    slack when interleaved with the heads loop).  Issues the
                    batched scratch-write DMAs after the 4th tile of each
                    l-half."""
                    p = hh // 2
                    lh, a = divmod(s, 4)
                    hls = range(2) if only_hl is None else (only_hl,)
                    lt = 4 * lh + a
                    l0 = 128 * lt
                    lsl = slice(l0, l0 + 128)
                    # QE for the needed m-range only (m <= l0+127),
                    # both m-banks in one 2-bank PSUM tile per head
                    banks = qe_banks(lt)
                    psms = {
                        hl: qe_ps.tile([128, 1024], f32, name="qe", tag="sc")
                        for hl in hls
                    }
                    for mh, (m0, w) in enumerate(banks):
                        for hl in hls:
                            rows = slice(64 * hl, 64 * (hl + 1))
                            nc.tensor.matmul(
                                psms[hl][:, 512 * mh : 512 * mh + w],
                                qhT[rows, p, lsl],
                                e2_sb[rows, m0 : m0 + w],
                                start=True,
                                stop=True,
                                tile_position=(64 * hl, 0),
                            )
                    # masked QE rows via one shifted-tri slab multiply
                    # per PSUM bank: slab[r, c] = ((c-512) <= r), so
                    # slab[:, 512-l0+m] = (m <= l0+r) = global tri
                    for hl in hls:
                        psm = psms[hl]
                        stripe = stripes[hl][lh][:, a, :]
                        if lt <= 3:
                            nc.vector.tensor_tensor(
                                stripe[:, 1 : 1 + l0 + 128],
                                psm[:, 0 : l0 + 128],
                                slab_sb[:, 512 - l0 : 640],
                                mult,
                            )
                        elif lt == 4:
                            nc.vector.tensor_tensor(
                                stripe[:, 1:513],
                                psm[:, 0:512],
                                slab_sb[:, 0:512],
                                mult,
                            )
                            nc.vector.tensor_tensor(
                                stripe[:, 513 : 1 + l0 + 128],
                                psm[:, 512 : 512 + l0 + 128 - 512],
                                slab_sb[:, 1024 - l0 : 640],
                                mult,
                            )
                        else:
                            # m < 512 is fully below the diagonal: copy
                            nc.vector.tensor_copy(
                                stripe[:, 1:513], psm[:, 0:512]
                            )
                            nc.vector.tensor_tensor(
                                stripe[:, 513 : 1 + l0 + 128],
                                psm[:, 512 : 512 + l0 + 128 - 512],
                                slab_sb[:, 1024 - l0 : 640],
                                mult,
                            )
                    if a == 3:
                        # one DMA per head for the 4 padded stripes (zero
                        # col/tails are persistent in the buffer).  Issued on
                        # the gpsimd SWDGE queue so the 16.8MB of scratch
                        # writes never queue ahead of the latency-critical
                        # srel reads / input loads on the sync queue.
                        for hl in hls:
                            dst = bass.AP(
                                tensor=scratch[hh + hl],
                                offset=512 * lh * PAD,
                                ap=[[PAD, 128], [128 * PAD, 4], [1, PAD]],
                            )
                            nc.gpsimd.dma_start(out=dst, in_=stripes[hl][lh])

                def srel_prefetch(h, lh):
                    """Allocate + issue the skew reads for (h, lh) early so
                    the in-order DMA queue overlaps them with compute."""
                    srel = srl.tile([128, 4, L], bf16, name="srel")
                    if lh == 0:
                        # low l-half: read only the nonzero jt span per lt
                        for a in range(4):
                            lt = a
                            nzj = [jt for jt in range(NLT)
                                   if srel_block_nonzero(lt, jt)]
                            j0, j1 = 128 * min(nzj), 128 * (max(nzj) + 1)
                            src = bass.AP(
                                tensor=scratch[h],
                                offset=(128 * lt + 1) * L + j0,
                                ap=[[L, 128], [1, j1 - j0]],
                            )
                            nc.sync.dma_start(out=srel[:, a, j0:j1], in_=src)
                    else:
                        # high l-half: dense, one batched DMA
                        src = bass.AP(
                            tensor=scratch[h],
                            offset=(512 * lh + 1) * L,
                            ap=[[L, 128], [128 * L, 4], [1, L]],
                        )
                        nc.sync.dma_start(out=srel, in_=src)
                    return srel

                def sc_tile(h, lh, t, srel, attnT_half):
                    """scores^T + Srel^T + exp for j-tiles 2t/2t+1 of one
                    l-half of head h (one 2-bank PSUM tile, one exp)."""
                    p, hl = divmod(h, 2)
                    rows = slice(64 * hl, 64 * (hl + 1))
                    tp = (64 * hl, 0)
                    lsl = slice(512 * lh, 512 * (lh + 1))
                    ps2 = sc_ps.tile([128, 1024], f32, name="sc", tag="sc")
                    for sub in range(2):
                        jt = 2 * t + sub
                        jsl = slice(128 * jt, 128 * (jt + 1))
                        c0 = 512 * sub
                        nzs = [
                            a for a in range(4)
                            if srel_block_nonzero(4 * lh + a, jt)
                        ]
                        # scores^T = kh qh^T for this (j-tile, l-half)
                        nc.tensor.matmul(
                            ps2[:, c0 : c0 + 512],
                            khT[rows, p, jsl],
                            qhT[rows, p, lsl],
                            start=True,
                            stop=(len(nzs) == 0),
                            tile_position=tp,
                        )
                        # += Srel^T via PE transpose-by-identity
                        for i, a in enumerate(nzs):
                            nc.tensor.matmul(
                                ps2[:, c0 + 128 * a : c0 + 128 * a + 128],
                                srel[:, a, jsl],
                                ident,
                                start=False,
                                stop=(i == len(nzs) - 1),
                            )
                    # one exp for both j-tiles (2 PSUM banks)
                    o_ap = attnT_half[:, 2 * t : 2 * t + 2, :]
                    p_ap = ps2[:, :]
                    p_view = bass.AP(
                        tensor=p_ap.tensor,
                        offset=p_ap.offset,
                        ap=[list(p_ap.ap)[0], [512, 2], [1, 512]],
                    )
                    nc.scalar.activation(o_ap, p_view, Exp, scale=0.125)

                def attnv_mm(st, lh, jt):
                    """One attn@V accumulation step for head st['h'] — a
                    single K=128 matmul per j-block.  (Splitting into two
                    co-streamed K=64 halves does NOT help: the PE drains one
                    output column per cycle through its single PSUM write
                    port, so PE time is set by output-column count — the
                    K=64 split emits every column twice.)  The ones column
                    of vh lands the softmax denominators Z on partition 64
                    of the same 1-bank PSUM tile; the normalize chain runs
                    inline at the last j-tile."""
                    h = st["h"]
                    if jt == 0:
                        st["cps"][lh] = ctx_ps.tile(
                            [HD + 1, 512], f32, name="cps", tag="cps"
                        )
                    cps = st["cps"][lh]
                    nc.tensor.matmul(
                        cps,
                        vh[:, jt, h, :],
                        st["halves"][lh][:, jt, :],
                        start=(jt == 0),
                        stop=(jt == NLT - 1),
                    )
                    if jt == NLT - 1:
                        p, hl = divmod(h, 2)
                        rows = slice(64 * hl, 64 * (hl + 1))
                        # Z to SBUF partition 0 first: the custom-DVE
                        # reciprocal op needs a plain SBUF operand
                        zq = zp.tile([1, 512], f32, name="zq")
                        nc.vector.tensor_copy(zq, cps[HD : HD + 1, :])
                        zqi = zp.tile([1, 512], f32, name="zqi")
                        nc.vector.reciprocal_approx_fast(zqi, zq)
                        zbc = zp.tile([64, 512], f32, name="zbc")
                        # broadcast across 64 partitions on the (idle)
                        # gpsimd engine — no DMA round trip
                        nc.gpsimd.partition_broadcast(zbc, zqi[0:1, :])
                        nc.vector.tensor_tensor(
                            ctxp[rows, p, 512 * lh : 512 * (lh + 1)],
                            cps[0:HD, :],
                            zbc,
                            mult,
                        )

                def op_tile(p, lt):
                    """One l-tile of pair p's output-projection partial:
                    out_p = ctx_pair_p @ Wo_p, drained to bf16 and written to
                    the per-pair DRAM partial.  Emitted interleaved into the
                    heads loop (pair p during head 2p+3; pair 3 in the tail)
                    so the old serial phase-6 tail disappears.  Mid-loop
                    drains go to the DVE (ScalarE is saturated by the exp
                    stream there); the tail pair alternates ScalarE/DVE."""
                    lsl = slice(128 * lt, 128 * (lt + 1))
                    ps = sc_ps.tile([128, 1024], f32, name="op", tag="sc")
                    for jh in range(2):
                        jsl = slice(512 * jh, 512 * (jh + 1))
                        nc.tensor.matmul(
                            ps[:, jsl],
                            ctxp[:, p, lsl],
                            wo_sb[p][:, jsl],
                            start=True,
                            stop=True,
                        )
                    obuf = ost.tile([128, D], bf16, name="o")
                    if p == 3 and lt % 2 == 0:
                        nc.scalar.copy(obuf, ps)
                    else:
                        nc.vector.tensor_copy(obuf, ps)
                    # out writes ride the scalar HWDGE queue (sync carries
                    # the srel reads, gpsimd the scratch writes)
                    nc.scalar.dma_start(out=out_d[p, lsl, :], in_=obuf)

                # ---- emission: projections, then a slot-interleaved scores/
                # attnV/stripes pipeline: each slot emits always-ready attn@V
                # matmuls of head h-1 first, then the scores tile, then a QE
                # l-tile of head h+2 — so a PSUM-rotation stall on any stream
                # is absorbed by ready work ahead of it in the in-order PE
                # queue and the HAM clock gate stays warm. ----
                for p in range(NPAIR):
                    proj_pair(wq_sb, qT, qhT, p)
                # QE pairs 0,1 run in this phase (interleaved with k-proj so
                # stripe WAR chains and scratch writes hide under PE work);
                # pairs 2,3 move into head 1/2 slots — that shortens the
                # serial lead-in AND gives each stripe-buffer reuse a full
                # head of slack over the previous pair's write-DMA drain.
                for s in range(8):
                    qe_part2(0, s)
                sr = None
                qe1_sched = {0: (0, 1, 2), 1: (3, 4, 5), 2: (6, 7)}
                for p in range(NPAIR):
                    proj_pair(wk_sb, kT, khT, p)
                    if p == 0:
                        # head-0 skew reads issue mid-proj (scratch 0 is
                        # already written), ahead of later stripe writes in
                        # the in-order sync queue
                        sr = [srel_prefetch(0, 0), srel_prefetch(0, 1)]
                    for s in qe1_sched.get(p, ()):
                        qe_part2(2, s)
                tin_blk.close()

                st_prev = None
                for h in range(H_LOC):
                    halves = hv0 if h == 0 else [
                        attT.tile([128, NLT, 512], bf16, name="attnT")
                        for _ in range(2)
                    ]
                    st = {"h": h, "halves": halves, "cps": {}}
                    sr_next = [None, None]
                    for s in range(8):
                        lh, t = divmod(s, 4)
                        # head 0 has no previous head's attn@V to interleave:
                        # fill its slots with the vh projection instead
                        if h == 0:
                            vh_tile(s)
                        # slot 4 starts the prev head's lh1 attn@V, whose ctx
                        # buffer recycles through lh0's normalize (emitted at
                        # slot 3) — lead with the ready scores tile there
                        if s == 4:
                            sc_tile(h, lh, t, sr[lh], halves[lh])
                        if st_prev is not None:
                            attnv_mm(st_prev, lh, 2 * t)
                            attnv_mm(st_prev, lh, 2 * t + 1)
                        if s != 4:
                            sc_tile(h, lh, t, sr[lh], halves[lh])
                        # QE for heads 4-7 streams one head per loop-head
                        # through heads 1-4 (scratch h+3 is read >=2 heads
                        # later), and the out-projection pairs through heads
                        # 5-7 — so every slot allocates exactly TWO big
                        # PSUM tiles (sc + one other) from the shared 3-buf
                        # pool, keeping one slot of rotation slack
                        if 1 <= h <= 4:
                            qe_part2(4 + 2 * ((h - 1) // 2), s,
                                     only_hl=(h - 1) % 2)
                        if h in (5, 6, 7):
                            op_tile(h - 5, s)
                        if s == 1 and h + 1 < H_LOC:
                            sr_next[0] = srel_prefetch(h + 1, 0)
                            sr_next[1] = srel_prefetch(h + 1, 1)
                    if h == 0:
                        # v inputs + matmul PSUM no longer needed; free them
                        # for the attn@V context banks
                        tinv_blk.close()
                        ctx_ps = outer2.enter_context(
                            tc.tile_pool(name="ctx_ps", bufs=2, space="PSUM")
                        )
                        attT = outer2.enter_context(
                            tc.tile_pool(name="attT", bufs=4)
                        )
                        ost = outer2.enter_context(
                            tc.tile_pool(name="ost", bufs=4)
                        )
                    st_prev = st
                    sr = sr_next
                # drain the last head's attn@V + pair 3's out-projection.
                # lh1 runs first so its ctx (l-tiles 4-7) lands at slot 3
                # and those op tiles overlap the lh0 attn@V in slots 4-7;
                # only the lh0 op tiles trail the last matmul
                for s in range(8):
                    lh = 1 - s // 4
                    t = s % 4
                    attnv_mm(st_prev, lh, 2 * t)
                    attnv_mm(st_prev, lh, 2 * t + 1)
                    if s >= 4:
                        op_tile(3, s)
                for lt in range(4):
                    op_tile(3, lt)

    nc.compile()
    return nc


TRACE = False
TRACE_KWARGS = {}
LAST_RESULT = None

_NC_CACHE = None


def _get_nc():
    global _NC_CACHE
    if _NC_CACHE is None:
        _NC_CACHE = _build_bass()
    return _NC_CACHE


def make_in_maps(k, v, q, E, Wk, Wv, Wq, Wo):
    """Host-side sharding: returns per-core input dicts."""
    eT = np.ascontiguousarray(E[MAX_SEQ - L :, :].T)  # [64, 1024]
    e2 = np.concatenate([eT, eT], axis=0).astype(BF16)  # [128, 1024]
    tri = (np.arange(128)[None, :] <= np.arange(128)[:, None]).astype(np.float32)
    slab = (
        (np.arange(640)[None, :] - 512) <= np.arange(128)[:, None]
    ).astype(BF16)
    qkvT = {}
    for b in range(B):
        qkvT[b] = (
            np.ascontiguousarray(np.asarray(q[b]).T).astype(BF16),
            np.ascontiguousarray(np.asarray(k[b]).T).astype(BF16),
            np.ascontiguousarray(np.asarray(v[b]).T).astype(BF16),
        )
    in_maps = []
    for core in range(NCORES):
        b, hg = divmod(core, 2)
        csl = slice(DG * hg, DG * (hg + 1))
        qTb, kTb, vTb = qkvT[b]
        in_maps.append(
            {
                "qT": qTb,
                "kT": kTb,
                "vT": vTb,
                "wq": np.ascontiguousarray(Wq[:, csl]).astype(BF16),
                "wk": np.ascontiguousarray(Wk[:, csl]).astype(BF16),
                "wv": np.ascontiguousarray(Wv[:, csl]).astype(BF16),
                "wo": np.ascontiguousarray(Wo[DG * hg : DG * (hg + 1), :]).astype(BF16),
                "e2": e2,
                "tri": tri,
                "slab": slab,
            }
        )
    return in_maps


def kernel(
    k,
    v,
    q,
    mask,
    E,
    Wk,
    bk,
    Wv,
    bv,
    Wq,
    bq,
    Wo,
    bo,
):
    k = np.asarray(k, np.float32)
    v = np.asarray(v, np.float32)
    q = np.asarray(q, np.float32)
    E = np.asarray(E, np.float32)
    Wk = np.asarray(Wk, np.float32)
    Wv = np.asarray(Wv, np.float32)
    Wq = np.asarray(Wq, np.float32)
    Wo = np.asarray(Wo, np.float32)
    mask = np.asarray(mask)
    assert bool(mask.all()), "kernel specialized for all-true mask"
    for bias in (bk, bv, bq):
        assert not np.any(np.asarray(bias)), "kernel specialized for zero qkv biases"
    bo = np.asarray(bo, np.float32)

    from concourse.bass_utils import run_bass_kernel_spmd

    nc = _get_nc()
    in_maps = make_in_maps(k, v, q, E, Wk, Wv, Wq, Wo)
    res = run_bass_kernel_spmd(
        nc, in_maps, core_ids=list(range(NCORES)), trace=TRACE, **TRACE_KWARGS
    )
    global LAST_RESULT
    LAST_RESULT = res
    out = np.zeros((B, L, D), np.float32)
    for core in range(NCORES):
        b = core // 2
        # per-pair bf16 partials: sum the 4 pairs in f32 on the host
        out[b] += np.asarray(res.results[core]["out"]).astype(np.float32).sum(axis=0)
    out += bo[None, None, :]
    return out

